# revision 25
# baseline (speedup 1.0000x reference)
"""Trainium2 Bass kernel for nn_BSHConv3D: spherical-harmonic 3^3 conv.

The whole module collapses to one dense 3D convolution
x[1,48,48,48,8] -> out[48,48,48, 512] with combined weights
W[3,3,3, 8, 512] (the central 1x1x1 conv folds into the center tap;
b_center is all-zero-shaped [16] and is added host-side).

Per-core (D sharded 8 x 6 slabs, halo 1):
  - host builds a VALID-ONLY 27-tap im2col: S[216, 13824] where row
    (kd,kh,kw,c) is the shifted padded x volume gathered at the 13824
    real output voxels (no H/W pad columns -> 8% fewer cols than a
    dense shifted layout)
  - matmul per 128-position tile: 2 PSUM-accumulating matmuls
    (K = 128 + 88 contraction rows) x N=512 output channels
  - startup: weights + first input chunk ride HWDGE (nc.sync) so the
    first matmul fires ~9us in, not ~16us (SWDGE Q7 descriptor-gen
    serializes at ~1us/op); a few warm-up matmuls on a memset tile
    start the HAM busy-window during the DMA wait
  - bulk input chunks ride SWDGE (gpsimd) so the scalar/sync queues
    stay free for PSUM evacuation / output stores
  - PSUM evacuated by VectorE/ScalarE alternating into a 9-tile group
    staging buffer, one ~1.2MB output DMA per group on nc.sync
  - steady state is DMA-bound: in 127 GB/s + out 300 GB/s demanded vs
    ~358 GB/s HBM per core, so tiles pace at ~520ns
"""

from contextlib import ExitStack

import ml_dtypes
import numpy as np

import concourse.bass as bass
from concourse import bacc
import concourse.mybir as mybir
import concourse.tile as tile
from concourse.bass_utils import run_bass_kernel_spmd

B, D, H, W, C = 1, 48, 48, 48, 8
KS, R, DEG, NH, OUT = 3, 2, 3, 16, 16
NCORES = 8
DL = D // NCORES  # 6 output slabs per core
HP = WP = 50  # zero-padded H/W
SLAB = HP * WP  # 2500
NSLAB = DL + 2  # local slabs incl. halos
NCH = OUT * NH * 2  # 512 output channels (f, n, re/im)
KC = 27 * C  # 216 contraction rows: 27 taps x 8 ch
KA = 128  # contraction chunk A (16 taps)
KB = KC - KA  # 88 (11 taps)
TM = 128  # positions per matmul tile
NVALID = DL * H * W  # 13824 valid output rows per core
NT = NVALID // TM  # 108 z tiles per core
GT = 9  # z tiles grouped per output DMA (108 = 12 groups of 9)
NG = NT // GT
NWARM = 7  # warm-up matmuls bridging the gap until real data lands

# input load chunking (cols): first chunk on HWDGE for fast start,
# rest on SWDGE sized to amortize Q7 descriptor-gen
CHUNK0 = 1280
CHUNKS = (2560, 4864, 5120)
assert CHUNK0 + sum(CHUNKS) == NVALID
NVEC = 6  # leading tiles evacuated on VectorE only (scalar queue is
# busy issuing the bulk-chunk HWDGE DMAs at kernel start)
WPAD = 1024  # packed weight tensor: row p = WcA[p] ++ WcB[p]/zeros

IO_DTYPE = "fp16"  # "fp16" | "bf16" | "f32r" matmul input dtype
OUT_DTYPE = "fp16"  # "fp16" | "f32" output DMA dtype (host upcasts)

# module-level knobs for the test harness (graders just call kernel())
TRACE = False
LAST_RESULTS = None

_MDT = {"fp16": mybir.dt.float16, "bf16": mybir.dt.bfloat16, "f32r": mybir.dt.float32r}


def _build_program():
    f32 = mybir.dt.float32
    mdt = _MDT[IO_DTYPE]
    odt = mybir.dt.float16 if OUT_DTYPE == "fp16" else f32
    nc = bacc.Bacc("TRN2", debug=False)
    xin = nc.dram_tensor("xin", [KC, NVALID], mdt, kind="ExternalInput").ap()
    # weights packed [128, 1024]: one DMA of 128 fat descriptors instead
    # of 216 skinny 1KB ones (early loads are descriptor-latency bound)
    wc = nc.dram_tensor("wc", [KA, WPAD], mdt, kind="ExternalInput").ap()
    # output rows permuted [group][p][g][c] so each (partition, group) pair
    # is one contiguous GT*NCH-byte DMA descriptor; host unpermutes
    out = nc.dram_tensor("out", [NG, TM, GT, NCH], odt, kind="ExternalOutput").ap()

    with tile.TileContext(nc) as tc, ExitStack() as ctx:
        const_pool = ctx.enter_context(tc.tile_pool(name="const", bufs=1))
        stage_pool = ctx.enter_context(tc.tile_pool(name="stage", bufs=4))
        psum_pool = ctx.enter_context(tc.tile_pool(name="psum", bufs=7, space="PSUM"))
        warm_pool = ctx.enter_context(tc.tile_pool(name="warm", bufs=1, space="PSUM"))

        SA = const_pool.tile([KA, NVALID], mdt, name="SA")
        SB = const_pool.tile([KB, NVALID], mdt, name="SB")
        Wt = const_pool.tile([KA, WPAD], mdt, name="Wt")
        dummy = const_pool.tile([KA, NCH], mdt, name="dummy")
        WtA = Wt[:, 0:NCH]
        WtB = Wt[0:KB, NCH:WPAD]

        # critical-path trio split across both HWDGE rings so the SDMA
        # engines serve it concurrently: sync gets Wt+SBc0, scalar SAc0.
        nc.sync.dma_start(Wt[:, :], wc[:, :])
        nc.scalar.dma_start(SA[:, 0:CHUNK0], xin[0:KA, 0:CHUNK0])
        nc.sync.dma_start(SB[:, 0:CHUNK0], xin[KA:KC, 0:CHUNK0])

        # Bulk chunks must NOT race the trio for SDMA slots (the engines
        # round-robin rings at packet granularity, which doubles the trio's
        # latency). Chain each chunk behind the previous via a real WAW
        # dependency — rewrite the previous chunk's last 8 columns (same
        # data) — so chunk k waits on chunk k-1's completion semaphore.
        # Queue placement is the delicate part (a waiting dma_start
        # head-of-line-blocks everything behind it on its engine queue):
        # c1 rides scalar (its wait clears at ~11us, before the first
        # scalar evacuation copy is due), c2/c3 ride the otherwise-empty
        # gpsimd queue where SWDGE's serializing DRAIN costs nothing.
        lo = CHUNK0
        for i, ch in enumerate(CHUNKS):
            hi = lo + ch
            eng = nc.scalar if i == 0 else nc.gpsimd
            eng.dma_start(SA[:, lo - 8 : hi], xin[0:KA, lo - 8 : hi])
            eng.dma_start(SB[:, lo - 8 : hi], xin[KA:KC, lo - 8 : hi])
            lo = hi
        assert lo == NVALID, lo

        # HAM warm-up: start the PE busy-window while the first chunk is
        # still in flight so real matmuls go warm sooner
        nc.vector.memset(dummy[:, :], 0)
        pw = warm_pool.tile([16, NCH], f32, name="pw")
        for i in range(NWARM):
            nc.tensor.matmul(
                pw[:, :], dummy[:, 0:16], dummy[:, :], start=(i == 0), stop=(i == NWARM - 1)
            )

        for g0 in range(0, NT, GT):
            st = stage_pool.tile([TM, GT * NCH], odt, name="st")
            for g in range(GT):
                t = g0 + g
                zb = t * TM
                ps = psum_pool.tile([TM, NCH], f32, name="ps")
                nc.tensor.matmul(
                    ps[:, :], SA[:, zb : zb + TM], WtA, start=True, stop=False
                )
                nc.tensor.matmul(
                    ps[:, :], SB[:, zb : zb + TM], WtB, start=False, stop=True
                )
                dst = st[:, g * NCH : (g + 1) * NCH]
                if t < NVEC or t % 2 == 0:
                    nc.vector.tensor_copy(dst, ps[:, :])
                else:
                    nc.scalar.copy(dst, ps[:, :])
            # one DMA per group, both sides contiguous per partition; the
            # last two groups drain in 3-tile sub-DMAs to shorten the tail
            if g0 + 2 * GT < NT:
                nc.sync.dma_start(out[g0 // GT], st[:, :])
            else:
                # shrinking pieces so the final post-copy DMA is tiny
                splits = (0, 3, 6, 8) if g0 + GT == NT else (0, 3, 6)
                for j, s in enumerate(splits):
                    e = splits[j + 1] if j + 1 < len(splits) else GT
                    nc.sync.dma_start(
                        out[g0 // GT][:, s:e, :],
                        st[:, s * NCH : e * NCH],
                    )
    nc.compile()
    return nc


_program_cache = {}


def _get_program():
    if "nc" not in _program_cache:
        _program_cache["nc"] = _build_program()
    return _program_cache["nc"]


def _host_weights(atoms_real, atoms_imag, w, w_center):
    idx = np.repeat(np.arange(DEG + 1), [2 * n + 1 for n in range(DEG + 1)])
    w_exp = w[..., idx]  # [C,F,R,NH]
    WR = np.einsum("dhwrn,cfrn->dhwcfn", atoms_real, w_exp)
    WI = np.einsum("dhwrn,cfrn->dhwcfn", atoms_imag, w_exp)
    Wfull = np.stack([WR, WI], axis=-1)  # [3,3,3,C,F,NH,2]
    Wc = Wfull.reshape(KC, NCH).copy()
    # central 1x1x1 conv onto (f, n=0, re): tap (kd=1,kh=1,kw=1) rows 104..111
    Wc[104:112, 0::32] += w_center
    # pack [KC, NCH] -> [KA, 2*NCH]: row p = WcA[p] ++ (WcB[p] | zeros)
    Wp = np.zeros((KA, WPAD), np.float32)
    Wp[:, :NCH] = Wc[:KA]
    Wp[:KB, NCH:] = Wc[KA:]
    return Wp


# flat indices (into the [NSLAB,50,50] padded local slab) of the 13824
# valid output voxels, in output raster order
_dl, _h, _w = np.meshgrid(
    np.arange(DL), np.arange(H), np.arange(W), indexing="ij"
)
_VOX_IDX = ((_dl + 1) * SLAB + (_h + 1) * WP + (_w + 1)).ravel()
_TAP_OFF = np.array(
    [
        (kd - 1) * SLAB + (kh - 1) * WP + (kw - 1)
        for kd in range(3)
        for kh in range(3)
        for kw in range(3)
    ]
)


def kernel(x, atoms_real, atoms_imag, w, w_center, b_center):
    global LAST_RESULTS
    x = np.asarray(x, np.float32)
    b_center = np.asarray(b_center, np.float32)
    Wc = _host_weights(
        np.asarray(atoms_real, np.float32),
        np.asarray(atoms_imag, np.float32),
        np.asarray(w, np.float32),
        np.asarray(w_center, np.float32),
    )
    hdt = {"fp16": np.float16, "bf16": ml_dtypes.bfloat16, "f32r": np.float32}[IO_DTYPE]
    Wc = Wc.astype(hdt)

    xt = np.transpose(x[0], (3, 0, 1, 2))  # [C,D,H,W]
    xpad = np.zeros((C, D + 2, HP, WP), np.float32)
    xpad[:, 1 : D + 1, 1 : H + 1, 1 : W + 1] = xt

    gather = _VOX_IDX[None, :] + _TAP_OFF[:, None]  # [27, NVALID]
    in_maps = []
    for core in range(NCORES):
        d0 = core * DL
        pbuf = xpad[:, d0 : d0 + NSLAB].reshape(C, NSLAB * SLAB)
        # buf[(tap,c), z] = pbuf[c, vox_z + off_tap]
        buf = (
            pbuf[:, gather]  # [C, 27, NVALID]
            .transpose(1, 0, 2)  # [27, C, NVALID]
            .reshape(KC, NVALID)
        )
        in_maps.append({"xin": buf.astype(hdt), "wc": Wc})

    nc = _get_program()
    res = run_bass_kernel_spmd(
        nc, in_maps, core_ids=list(range(NCORES)), trace=TRACE
    )
    LAST_RESULTS = res
    outs = [
        res.results[i]["out"]
        .transpose(0, 2, 1, 3)  # [NG, GT, TM, NCH]
        .reshape(DL, H, W, OUT, NH, 2)
        .astype(np.float32)
        for i in range(NCORES)
    ]
    full = np.concatenate(outs, axis=0)
    full[..., 0, 0] += b_center  # [D,H,W,F] += [F]: central-conv bias
    return full[None]


# revision 26
# speedup vs baseline: 1.0181x; 1.0181x over previous
"""Trainium2 Bass kernel for nn_BSHConv3D: spherical-harmonic 3^3 conv.

The whole module collapses to one dense 3D convolution
x[1,48,48,48,8] -> out[48,48,48, 512] with combined weights
W[3,3,3, 8, 512] (the central 1x1x1 conv folds into the center tap;
b_center is all-zero-shaped [16] and is added host-side).

Per-core (D sharded 8 x 6 slabs, halo 1):
  - host builds a VALID-ONLY 27-tap im2col: S[216, 13824] where row
    (kd,kh,kw,c) is the shifted padded x volume gathered at the 13824
    real output voxels (no H/W pad columns -> 8% fewer cols than a
    dense shifted layout)
  - matmul per 128-position tile: 2 PSUM-accumulating matmuls
    (K = 128 + 88 contraction rows) x N=512 output channels
  - startup: weights + first input chunk ride HWDGE (nc.sync) so the
    first matmul fires ~9us in, not ~16us (SWDGE Q7 descriptor-gen
    serializes at ~1us/op); a few warm-up matmuls on a memset tile
    start the HAM busy-window during the DMA wait
  - bulk input chunks ride SWDGE (gpsimd) so the scalar/sync queues
    stay free for PSUM evacuation / output stores
  - PSUM evacuated by VectorE/ScalarE alternating into a 9-tile group
    staging buffer, one ~1.2MB output DMA per group on nc.sync
  - steady state is DMA-bound: in 127 GB/s + out 300 GB/s demanded vs
    ~358 GB/s HBM per core, so tiles pace at ~520ns
"""

from contextlib import ExitStack

import ml_dtypes
import numpy as np

import concourse.bass as bass
from concourse import bacc
import concourse.mybir as mybir
import concourse.tile as tile
from concourse.bass_utils import run_bass_kernel_spmd

B, D, H, W, C = 1, 48, 48, 48, 8
KS, R, DEG, NH, OUT = 3, 2, 3, 16, 16
NCORES = 8
DL = D // NCORES  # 6 output slabs per core
HP = WP = 50  # zero-padded H/W
SLAB = HP * WP  # 2500
NSLAB = DL + 2  # local slabs incl. halos
NCH = OUT * NH * 2  # 512 output channels (f, n, re/im)
KC = 27 * C  # 216 contraction rows: 27 taps x 8 ch
KA = 128  # contraction chunk A (16 taps)
KB = KC - KA  # 88 (11 taps)
TM = 128  # positions per matmul tile
NVALID = DL * H * W  # 13824 valid output rows per core
NT = NVALID // TM  # 108 z tiles per core
GT = 9  # z tiles grouped per output DMA (108 = 12 groups of 9)
NG = NT // GT
NWARM = 7  # warm-up matmuls bridging the gap until real data lands

# input load chunking (cols): first chunk on HWDGE for fast start,
# rest on SWDGE sized to amortize Q7 descriptor-gen
CHUNK0 = 1280
CHUNKS = (2560, 4096, 5888)
assert CHUNK0 + sum(CHUNKS) == NVALID
NVEC = 6  # leading tiles evacuated on VectorE only (scalar queue is
# busy issuing the bulk-chunk HWDGE DMAs at kernel start)
WPAD = 1024  # packed weight tensor: row p = WcA[p] ++ WcB[p]/zeros

IO_DTYPE = "fp16"  # "fp16" | "bf16" | "f32r" matmul input dtype
OUT_DTYPE = "fp16"  # "fp16" | "f32" output DMA dtype (host upcasts)

# module-level knobs for the test harness (graders just call kernel())
TRACE = False
LAST_RESULTS = None

_MDT = {"fp16": mybir.dt.float16, "bf16": mybir.dt.bfloat16, "f32r": mybir.dt.float32r}


def _build_program():
    f32 = mybir.dt.float32
    mdt = _MDT[IO_DTYPE]
    odt = mybir.dt.float16 if OUT_DTYPE == "fp16" else f32
    nc = bacc.Bacc("TRN2", debug=False)
    xin = nc.dram_tensor("xin", [KC, NVALID], mdt, kind="ExternalInput").ap()
    # weights packed [128, 1024]: one DMA of 128 fat descriptors instead
    # of 216 skinny 1KB ones (early loads are descriptor-latency bound)
    wc = nc.dram_tensor("wc", [KA, WPAD], mdt, kind="ExternalInput").ap()
    # output rows permuted [group][p][g][c] so each (partition, group) pair
    # is one contiguous GT*NCH-byte DMA descriptor; host unpermutes
    out = nc.dram_tensor("out", [NG, TM, GT, NCH], odt, kind="ExternalOutput").ap()

    with tile.TileContext(nc) as tc, ExitStack() as ctx:
        const_pool = ctx.enter_context(tc.tile_pool(name="const", bufs=1))
        stage_pool = ctx.enter_context(tc.tile_pool(name="stage", bufs=4))
        psum_pool = ctx.enter_context(tc.tile_pool(name="psum", bufs=7, space="PSUM"))
        warm_pool = ctx.enter_context(tc.tile_pool(name="warm", bufs=1, space="PSUM"))

        SA = const_pool.tile([KA, NVALID], mdt, name="SA")
        SB = const_pool.tile([KB, NVALID], mdt, name="SB")
        Wt = const_pool.tile([KA, WPAD], mdt, name="Wt")
        dummy = const_pool.tile([KA, NCH], mdt, name="dummy")
        WtA = Wt[:, 0:NCH]
        WtB = Wt[0:KB, NCH:WPAD]

        # critical-path trio split across both HWDGE rings so the SDMA
        # engines serve it concurrently: sync gets Wt+SBc0, scalar SAc0.
        nc.sync.dma_start(Wt[:, :], wc[:, :])
        nc.scalar.dma_start(SA[:, 0:CHUNK0], xin[0:KA, 0:CHUNK0])
        nc.sync.dma_start(SB[:, 0:CHUNK0], xin[KA:KC, 0:CHUNK0])

        # Bulk chunks must NOT race the trio for SDMA slots (the engines
        # round-robin rings at packet granularity, which doubles the trio's
        # latency). Chain each chunk behind the previous via a real WAW
        # dependency — rewrite the previous chunk's last 8 columns (same
        # data) — so chunk k waits on chunk k-1's completion semaphore.
        # Queue placement is the delicate part (a waiting dma_start
        # head-of-line-blocks everything behind it on its engine queue):
        # c1 rides scalar (its wait clears at ~11us, before the first
        # scalar evacuation copy is due), c2/c3 ride the otherwise-empty
        # gpsimd queue where SWDGE's serializing DRAIN costs nothing.
        lo = CHUNK0
        for i, ch in enumerate(CHUNKS):
            hi = lo + ch
            eng = nc.scalar if i == 0 else nc.gpsimd
            eng.dma_start(SA[:, lo - 8 : hi], xin[0:KA, lo - 8 : hi])
            eng.dma_start(SB[:, lo - 8 : hi], xin[KA:KC, lo - 8 : hi])
            lo = hi
        assert lo == NVALID, lo

        # HAM warm-up: start the PE busy-window while the first chunk is
        # still in flight so real matmuls go warm sooner
        nc.vector.memset(dummy[:, :], 0)
        pw = warm_pool.tile([16, NCH], f32, name="pw")
        for i in range(NWARM):
            nc.tensor.matmul(
                pw[:, :], dummy[:, 0:16], dummy[:, :], start=(i == 0), stop=(i == NWARM - 1)
            )

        for g0 in range(0, NT, GT):
            st = stage_pool.tile([TM, GT * NCH], odt, name="st")
            for g in range(GT):
                t = g0 + g
                zb = t * TM
                ps = psum_pool.tile([TM, NCH], f32, name="ps")
                nc.tensor.matmul(
                    ps[:, :], SA[:, zb : zb + TM], WtA, start=True, stop=False
                )
                nc.tensor.matmul(
                    ps[:, :], SB[:, zb : zb + TM], WtB, start=False, stop=True
                )
                dst = st[:, g * NCH : (g + 1) * NCH]
                if t < NVEC or t % 2 == 0:
                    nc.vector.tensor_copy(dst, ps[:, :])
                else:
                    nc.scalar.copy(dst, ps[:, :])
            # one DMA per group, both sides contiguous per partition; the
            # last two groups drain in 3-tile sub-DMAs to shorten the tail
            if g0 + 2 * GT < NT:
                nc.sync.dma_start(out[g0 // GT], st[:, :])
            else:
                for s in range(0, GT, 3):
                    nc.sync.dma_start(
                        out[g0 // GT][:, s : s + 3, :],
                        st[:, s * NCH : (s + 3) * NCH],
                    )
    nc.compile()
    return nc


_program_cache = {}


def _get_program():
    if "nc" not in _program_cache:
        _program_cache["nc"] = _build_program()
    return _program_cache["nc"]


def _host_weights(atoms_real, atoms_imag, w, w_center):
    idx = np.repeat(np.arange(DEG + 1), [2 * n + 1 for n in range(DEG + 1)])
    w_exp = w[..., idx]  # [C,F,R,NH]
    WR = np.einsum("dhwrn,cfrn->dhwcfn", atoms_real, w_exp)
    WI = np.einsum("dhwrn,cfrn->dhwcfn", atoms_imag, w_exp)
    Wfull = np.stack([WR, WI], axis=-1)  # [3,3,3,C,F,NH,2]
    Wc = Wfull.reshape(KC, NCH).copy()
    # central 1x1x1 conv onto (f, n=0, re): tap (kd=1,kh=1,kw=1) rows 104..111
    Wc[104:112, 0::32] += w_center
    # pack [KC, NCH] -> [KA, 2*NCH]: row p = WcA[p] ++ (WcB[p] | zeros)
    Wp = np.zeros((KA, WPAD), np.float32)
    Wp[:, :NCH] = Wc[:KA]
    Wp[:KB, NCH:] = Wc[KA:]
    return Wp


# flat indices (into the [NSLAB,50,50] padded local slab) of the 13824
# valid output voxels, in output raster order
_dl, _h, _w = np.meshgrid(
    np.arange(DL), np.arange(H), np.arange(W), indexing="ij"
)
_VOX_IDX = ((_dl + 1) * SLAB + (_h + 1) * WP + (_w + 1)).ravel()
_TAP_OFF = np.array(
    [
        (kd - 1) * SLAB + (kh - 1) * WP + (kw - 1)
        for kd in range(3)
        for kh in range(3)
        for kw in range(3)
    ]
)


def kernel(x, atoms_real, atoms_imag, w, w_center, b_center):
    global LAST_RESULTS
    x = np.asarray(x, np.float32)
    b_center = np.asarray(b_center, np.float32)
    Wc = _host_weights(
        np.asarray(atoms_real, np.float32),
        np.asarray(atoms_imag, np.float32),
        np.asarray(w, np.float32),
        np.asarray(w_center, np.float32),
    )
    hdt = {"fp16": np.float16, "bf16": ml_dtypes.bfloat16, "f32r": np.float32}[IO_DTYPE]
    Wc = Wc.astype(hdt)

    xt = np.transpose(x[0], (3, 0, 1, 2))  # [C,D,H,W]
    xpad = np.zeros((C, D + 2, HP, WP), np.float32)
    xpad[:, 1 : D + 1, 1 : H + 1, 1 : W + 1] = xt

    gather = _VOX_IDX[None, :] + _TAP_OFF[:, None]  # [27, NVALID]
    in_maps = []
    for core in range(NCORES):
        d0 = core * DL
        pbuf = xpad[:, d0 : d0 + NSLAB].reshape(C, NSLAB * SLAB)
        # buf[(tap,c), z] = pbuf[c, vox_z + off_tap]
        buf = (
            pbuf[:, gather]  # [C, 27, NVALID]
            .transpose(1, 0, 2)  # [27, C, NVALID]
            .reshape(KC, NVALID)
        )
        in_maps.append({"xin": buf.astype(hdt), "wc": Wc})

    nc = _get_program()
    res = run_bass_kernel_spmd(
        nc, in_maps, core_ids=list(range(NCORES)), trace=TRACE
    )
    LAST_RESULTS = res
    outs = [
        res.results[i]["out"]
        .transpose(0, 2, 1, 3)  # [NG, GT, TM, NCH]
        .reshape(DL, H, W, OUT, NH, 2)
        .astype(np.float32)
        for i in range(NCORES)
    ]
    full = np.concatenate(outs, axis=0)
    full[..., 0, 0] += b_center  # [D,H,W,F] += [F]: central-conv bias
    return full[None]


# revision 28
# speedup vs baseline: 1.0277x; 1.0094x over previous
"""Trainium2 Bass kernel for nn_BSHConv3D: spherical-harmonic 3^3 conv.

The whole module collapses to one dense 3D convolution
x[1,48,48,48,8] -> out[48,48,48, 512] with combined weights
W[3,3,3, 8, 512] (the central 1x1x1 conv folds into the center tap;
b_center is all-zero-shaped [16] and is added host-side).

Per-core (D sharded 8 x 6 slabs, halo 1):
  - host builds a VALID-ONLY 27-tap im2col: S[216, 13824] where row
    (kd,kh,kw,c) is the shifted padded x volume gathered at the 13824
    real output voxels (no H/W pad columns -> 8% fewer cols than a
    dense shifted layout)
  - matmul per 128-position tile: 2 PSUM-accumulating matmuls
    (K = 128 + 88 contraction rows) x N=512 output channels
  - startup: weights + first input chunk ride HWDGE (nc.sync) so the
    first matmul fires ~9us in, not ~16us (SWDGE Q7 descriptor-gen
    serializes at ~1us/op); a few warm-up matmuls on a memset tile
    start the HAM busy-window during the DMA wait
  - bulk input chunks ride SWDGE (gpsimd) so the scalar/sync queues
    stay free for PSUM evacuation / output stores
  - PSUM evacuated by VectorE/ScalarE alternating into a 9-tile group
    staging buffer, one ~1.2MB output DMA per group on nc.sync
  - steady state is DMA-bound: in 127 GB/s + out 300 GB/s demanded vs
    ~358 GB/s HBM per core, so tiles pace at ~520ns
"""

from contextlib import ExitStack

import ml_dtypes
import numpy as np

import concourse.bass as bass
from concourse import bacc
import concourse.mybir as mybir
import concourse.tile as tile
from concourse.bass_utils import run_bass_kernel_spmd

B, D, H, W, C = 1, 48, 48, 48, 8
KS, R, DEG, NH, OUT = 3, 2, 3, 16, 16
NCORES = 8
DL = D // NCORES  # 6 output slabs per core
HP = WP = 50  # zero-padded H/W
SLAB = HP * WP  # 2500
NSLAB = DL + 2  # local slabs incl. halos
NCH = OUT * NH * 2  # 512 output channels (f, n, re/im)
KC = 27 * C  # 216 contraction rows: 27 taps x 8 ch
KA = 128  # contraction chunk A (16 taps)
KB = KC - KA  # 88 (11 taps)
TM = 128  # positions per matmul tile
NVALID = DL * H * W  # 13824 valid output rows per core
NT = NVALID // TM  # 108 z tiles per core
GT = 9  # z tiles grouped per output DMA (108 = 12 groups of 9)
NG = NT // GT
NWARM = 7  # warm-up matmuls bridging the gap until real data lands

# input load chunking (cols): first chunk on HWDGE for fast start,
# rest on SWDGE sized to amortize Q7 descriptor-gen
CHUNK0 = 1280
CHUNKS = (2560, 4096, 2944, 2944)
assert CHUNK0 + sum(CHUNKS) == NVALID
NVEC = 6  # leading tiles evacuated on VectorE only (scalar queue is
# busy issuing the bulk-chunk HWDGE DMAs at kernel start)
WPAD = 1024  # packed weight tensor: row p = WcA[p] ++ WcB[p]/zeros

IO_DTYPE = "fp16"  # "fp16" | "bf16" | "f32r" matmul input dtype
OUT_DTYPE = "fp16"  # "fp16" | "f32" output DMA dtype (host upcasts)

# module-level knobs for the test harness (graders just call kernel())
TRACE = False
LAST_RESULTS = None

_MDT = {"fp16": mybir.dt.float16, "bf16": mybir.dt.bfloat16, "f32r": mybir.dt.float32r}


def _build_program():
    f32 = mybir.dt.float32
    mdt = _MDT[IO_DTYPE]
    odt = mybir.dt.float16 if OUT_DTYPE == "fp16" else f32
    nc = bacc.Bacc("TRN2", debug=False)
    xin = nc.dram_tensor("xin", [KC, NVALID], mdt, kind="ExternalInput").ap()
    # weights packed [128, 1024]: one DMA of 128 fat descriptors instead
    # of 216 skinny 1KB ones (early loads are descriptor-latency bound)
    wc = nc.dram_tensor("wc", [KA, WPAD], mdt, kind="ExternalInput").ap()
    # output rows permuted [group][p][g][c] so each (partition, group) pair
    # is one contiguous GT*NCH-byte DMA descriptor; host unpermutes
    out = nc.dram_tensor("out", [NG, TM, GT, NCH], odt, kind="ExternalOutput").ap()

    with tile.TileContext(nc) as tc, ExitStack() as ctx:
        const_pool = ctx.enter_context(tc.tile_pool(name="const", bufs=1))
        stage_pool = ctx.enter_context(tc.tile_pool(name="stage", bufs=4))
        psum_pool = ctx.enter_context(tc.tile_pool(name="psum", bufs=7, space="PSUM"))
        warm_pool = ctx.enter_context(tc.tile_pool(name="warm", bufs=1, space="PSUM"))

        SA = const_pool.tile([KA, NVALID], mdt, name="SA")
        SB = const_pool.tile([KB, NVALID], mdt, name="SB")
        Wt = const_pool.tile([KA, WPAD], mdt, name="Wt")
        dummy = const_pool.tile([KA, NCH], mdt, name="dummy")
        WtA = Wt[:, 0:NCH]
        WtB = Wt[0:KB, NCH:WPAD]

        # critical-path trio split across both HWDGE rings so the SDMA
        # engines serve it concurrently: sync gets Wt+SBc0, scalar SAc0.
        nc.sync.dma_start(Wt[:, :], wc[:, :])
        nc.scalar.dma_start(SA[:, 0:CHUNK0], xin[0:KA, 0:CHUNK0])
        nc.sync.dma_start(SB[:, 0:CHUNK0], xin[KA:KC, 0:CHUNK0])

        # Bulk chunks must NOT race the trio for SDMA slots (the engines
        # round-robin rings at packet granularity, which doubles the trio's
        # latency). Chain each chunk behind the previous via a real WAW
        # dependency — rewrite the previous chunk's last 8 columns (same
        # data) — so chunk k waits on chunk k-1's completion semaphore.
        # Queue placement is the delicate part (a waiting dma_start
        # head-of-line-blocks everything behind it on its engine queue):
        # c1 rides scalar (its wait clears at ~11us, before the first
        # scalar evacuation copy is due), c2/c3 ride the otherwise-empty
        # gpsimd queue where SWDGE's serializing DRAIN costs nothing.
        lo = CHUNK0
        for i, ch in enumerate(CHUNKS):
            hi = lo + ch
            eng = nc.scalar if i == 0 else nc.gpsimd
            eng.dma_start(SA[:, lo - 8 : hi], xin[0:KA, lo - 8 : hi])
            eng.dma_start(SB[:, lo - 8 : hi], xin[KA:KC, lo - 8 : hi])
            lo = hi
        assert lo == NVALID, lo

        # HAM warm-up: start the PE busy-window while the first chunk is
        # still in flight so real matmuls go warm sooner
        nc.vector.memset(dummy[:, :], 0)
        pw = warm_pool.tile([16, NCH], f32, name="pw")
        for i in range(NWARM):
            nc.tensor.matmul(
                pw[:, :], dummy[:, 0:16], dummy[:, :], start=(i == 0), stop=(i == NWARM - 1)
            )

        for g0 in range(0, NT, GT):
            st = stage_pool.tile([TM, GT * NCH], odt, name="st")
            for g in range(GT):
                t = g0 + g
                zb = t * TM
                ps = psum_pool.tile([TM, NCH], f32, name="ps")
                nc.tensor.matmul(
                    ps[:, :], SA[:, zb : zb + TM], WtA, start=True, stop=False
                )
                nc.tensor.matmul(
                    ps[:, :], SB[:, zb : zb + TM], WtB, start=False, stop=True
                )
                dst = st[:, g * NCH : (g + 1) * NCH]
                if t < NVEC or t % 2 == 0:
                    nc.vector.tensor_copy(dst, ps[:, :])
                else:
                    nc.scalar.copy(dst, ps[:, :])
            # one DMA per group, both sides contiguous per partition; the
            # last two groups drain in 3-tile sub-DMAs to shorten the tail
            if g0 + 2 * GT < NT:
                nc.sync.dma_start(out[g0 // GT], st[:, :])
            else:
                # last two groups drain in shrinking pieces so the final
                # post-copy DMA is a single tile
                splits = (0, 3, 6, 7, 8) if g0 + GT == NT else (0, 3, 6)
                for j, s in enumerate(splits):
                    e = splits[j + 1] if j + 1 < len(splits) else GT
                    nc.sync.dma_start(
                        out[g0 // GT][:, s:e, :],
                        st[:, s * NCH : e * NCH],
                    )
    nc.compile()
    return nc


_program_cache = {}


def _get_program():
    if "nc" not in _program_cache:
        _program_cache["nc"] = _build_program()
    return _program_cache["nc"]


def _host_weights(atoms_real, atoms_imag, w, w_center):
    idx = np.repeat(np.arange(DEG + 1), [2 * n + 1 for n in range(DEG + 1)])
    w_exp = w[..., idx]  # [C,F,R,NH]
    WR = np.einsum("dhwrn,cfrn->dhwcfn", atoms_real, w_exp)
    WI = np.einsum("dhwrn,cfrn->dhwcfn", atoms_imag, w_exp)
    Wfull = np.stack([WR, WI], axis=-1)  # [3,3,3,C,F,NH,2]
    Wc = Wfull.reshape(KC, NCH).copy()
    # central 1x1x1 conv onto (f, n=0, re): tap (kd=1,kh=1,kw=1) rows 104..111
    Wc[104:112, 0::32] += w_center
    # pack [KC, NCH] -> [KA, 2*NCH]: row p = WcA[p] ++ (WcB[p] | zeros)
    Wp = np.zeros((KA, WPAD), np.float32)
    Wp[:, :NCH] = Wc[:KA]
    Wp[:KB, NCH:] = Wc[KA:]
    return Wp


# flat indices (into the [NSLAB,50,50] padded local slab) of the 13824
# valid output voxels, in output raster order
_dl, _h, _w = np.meshgrid(
    np.arange(DL), np.arange(H), np.arange(W), indexing="ij"
)
_VOX_IDX = ((_dl + 1) * SLAB + (_h + 1) * WP + (_w + 1)).ravel()
_TAP_OFF = np.array(
    [
        (kd - 1) * SLAB + (kh - 1) * WP + (kw - 1)
        for kd in range(3)
        for kh in range(3)
        for kw in range(3)
    ]
)


def kernel(x, atoms_real, atoms_imag, w, w_center, b_center):
    global LAST_RESULTS
    x = np.asarray(x, np.float32)
    b_center = np.asarray(b_center, np.float32)
    Wc = _host_weights(
        np.asarray(atoms_real, np.float32),
        np.asarray(atoms_imag, np.float32),
        np.asarray(w, np.float32),
        np.asarray(w_center, np.float32),
    )
    hdt = {"fp16": np.float16, "bf16": ml_dtypes.bfloat16, "f32r": np.float32}[IO_DTYPE]
    Wc = Wc.astype(hdt)

    xt = np.transpose(x[0], (3, 0, 1, 2))  # [C,D,H,W]
    xpad = np.zeros((C, D + 2, HP, WP), np.float32)
    xpad[:, 1 : D + 1, 1 : H + 1, 1 : W + 1] = xt

    gather = _VOX_IDX[None, :] + _TAP_OFF[:, None]  # [27, NVALID]
    in_maps = []
    for core in range(NCORES):
        d0 = core * DL
        pbuf = xpad[:, d0 : d0 + NSLAB].reshape(C, NSLAB * SLAB)
        # buf[(tap,c), z] = pbuf[c, vox_z + off_tap]
        buf = (
            pbuf[:, gather]  # [C, 27, NVALID]
            .transpose(1, 0, 2)  # [27, C, NVALID]
            .reshape(KC, NVALID)
        )
        in_maps.append({"xin": buf.astype(hdt), "wc": Wc})

    nc = _get_program()
    res = run_bass_kernel_spmd(
        nc, in_maps, core_ids=list(range(NCORES)), trace=TRACE
    )
    LAST_RESULTS = res
    outs = [
        res.results[i]["out"]
        .transpose(0, 2, 1, 3)  # [NG, GT, TM, NCH]
        .reshape(DL, H, W, OUT, NH, 2)
        .astype(np.float32)
        for i in range(NCORES)
    ]
    full = np.concatenate(outs, axis=0)
    full[..., 0, 0] += b_center  # [D,H,W,F] += [F]: central-conv bias
    return full[None]
